# revision 44
# baseline (speedup 1.0000x reference)
"""Trainium2 Bass kernel for causal self-attention (T=2048, C=1024, NH=16).

Strategy (tensor-parallel over heads, 2 heads/core on 8 cores):
  - Host pre-packs x, w_attn-slice, w_proj into the exact SBUF layouts so
    every DMA is a [128, N] fully-contiguous transfer (cheap descriptor
    generation; strided rearranges cost ~3us each on the Sync engine).
  - DMA issue order follows need order: wqkv first, then x quarter 0 as 8
    per-fo chunks (the first qkv matmul group only needs chunk 0), then
    the rest; wproj (needed last) is issued last.
  - qkv block nt=0 runs f-major with three concurrent PSUM accumulators so
    matmuls start as soon as the first x chunk lands; later blocks run
    g-major.  Attention block bq is emitted one qkv block behind so the
    Scalar-engine exp work overlaps qkv matmuls.
  - All matmuls use the full 128x128 PE array (q/k zero-padded per head,
    v padded to 128 columns): the PE HAM clock-gate watches array
    *activity*; low-occupancy matmuls read as idle and re-throttle the
    clock to 4/8.
  - att_T = k @ q.T tiles ([t_k, t_q]) so softmax's denominator comes for
    free from an appended ones-column on v.  Softmax without
    max-subtraction (inputs bounded, |att| < 8).  exp runs only on the
    causally-valid column range of diagonal tiles; the per-block
    triangular mask is applied in-place on GpSimd (affine_select), and
    the q@k / att@v streams skip fully-masked columns.
  - The reference's bug-faithful reshape (NH,T,HD)->swap(1,2)->(T,C) makes
    the output row-parallel over heads: each core produces 256 full output
    rows; host concatenates, no collective.
  - The output projection is split: the first half of its contraction
    (tau chunks 0..3, complete after attention block 2) is computed during
    the Scalar-bound attention block 3 and spilled to SBUF, keeping the
    PE busy; the rest runs after.
"""
import math
import os

import numpy as np

import concourse.bass as bass
import concourse.bacc as bacc
import concourse.mybir as mybir
import concourse.tile as tile
from concourse import bass_utils
from concourse.masks import make_identity

T, C, NH, HD = 2048, 1024, 16, 64
P = 128
NCORES = 8
HPC = 2          # heads per core
F32 = mybir.dt.float32
MMDT = mybir.dt.bfloat16  # matmul input dtype
EXPF = mybir.ActivationFunctionType.Exp


def _to_mm(a):
    import ml_dtypes
    return np.ascontiguousarray(np.asarray(a, dtype=np.float32).astype(ml_dtypes.bfloat16))


def build_nc():
    nc = bacc.Bacc(trn_type="TRN2", target_bir_lowering=False)

    xq_d = nc.dram_tensor("xq", [P, 4, 8 * 512], MMDT, kind="ExternalInput")
    wqkv_d = nc.dram_tensor("wqkv", [P, 8 * 384], MMDT, kind="ExternalInput")
    bqkv_d = nc.dram_tensor("bqkv", [P, 3], F32, kind="ExternalInput")
    wproj_d = nc.dram_tensor("wproj", [P, 8 * 1024], MMDT, kind="ExternalInput")
    bproj_d = nc.dram_tensor("bproj", [1, C], F32, kind="ExternalInput")
    out_d = nc.dram_tensor("out", [2 * P, C], F32, kind="ExternalOutput")

    from contextlib import ExitStack

    with tile.TileContext(nc) as tc, ExitStack() as stack:
        consts = stack.enter_context(tc.tile_pool(name="consts", bufs=1))
        wpool = stack.enter_context(tc.tile_pool(name="wpool", bufs=1))
        main = stack.enter_context(tc.tile_pool(name="main", bufs=1))
        ps_q = stack.enter_context(tc.tile_pool(name="ps_q", bufs=2, space="PSUM"))
        ps_att = stack.enter_context(tc.tile_pool(name="ps_att", bufs=2, space="PSUM"))
        ps_y = stack.enter_context(tc.tile_pool(name="ps_y", bufs=2, space="PSUM"))

        # ---- DMAs split across both HWDGE queues (Sync + Scalar), issued
        # in need order; x quarter 1 gets 4 queue slots so it lands before
        # qkv block 1 needs it ----
        wqkv_s = wpool.tile([P, 8, 384], MMDT)
        xT_s = wpool.tile([P, 4, 8 * 512], MMDT)
        xr = xq_d.ap()
        wr = wqkv_d.ap().rearrange("p (f c) -> p f c", f=8)
        for lo, hi in ((0, 1), (1, 2), (2, 4), (4, 6), (6, 8)):
            nc.scalar.dma_start(out=wqkv_s[:, lo:hi, :], in_=wr[:, lo:hi, :])
            nc.sync.dma_start(out=xT_s[:, 0, 512 * lo:512 * hi],
                              in_=xr[:, 0, 512 * lo:512 * hi])
        for i in range(4):
            eng = nc.sync if i % 2 == 0 else nc.scalar
            eng.dma_start(out=xT_s[:, 1, 1024 * i:1024 * (i + 1)],
                          in_=xr[:, 1, 1024 * i:1024 * (i + 1)])
        bqkv_s = consts.tile([P, 3], F32)
        nc.scalar.dma_start(out=bqkv_s, in_=bqkv_d.ap())
        for qtr in range(2, 4):
            nc.sync.dma_start(out=xT_s[:, qtr, 0:2048], in_=xr[:, qtr, 0:2048])
            nc.scalar.dma_start(out=xT_s[:, qtr, 2048:4096], in_=xr[:, qtr, 2048:4096])
        wproj_s = wpool.tile([P, 8, 1024], MMDT)
        nc.sync.dma_start(out=wproj_s, in_=wproj_d.ap())
        bproj_bc = consts.tile([P, C], F32)
        bp = bproj_d.ap()
        bp_bcast = bass.AP(tensor=bp.tensor, offset=bp.offset,
                           ap=[[0, P]] + list(bp.ap[1:]))
        nc.scalar.dma_start(out=bproj_bc, in_=bp_bcast)

        # ---- constants / padded state ----
        with nc.named_scope("setup"):
            # PE warm-up: the HAM clock gate starts at K=4/8 and needs
            # ~3.4us of sustained full-array activity to release; burn that
            # time on zero matmuls while the input DMAs are still in flight
            # so the real matmuls start at full clock
            wu = consts.tile([P, 640], MMDT)
            nc.vector.memset(wu, 0.0)
            ps_wu = ps_q.tile([P, 512], F32, name="ps_wu", tag="mm")
            for i in range(8):
                nc.tensor.matmul(
                    ps_wu,
                    lhsT=wu[:, 0:P],
                    rhs=wu[:, P:640],
                    start=(i == 0),
                    stop=(i == 7),
                )
            # bf16 identity: keeps every PE transpose in bf16 so the fp32-HI
            # FWL-disable workaround never triggers
            identity = consts.tile([P, P], MMDT)
            make_identity(nc, identity)

        # q/k for both heads in single tiles: head A channels on partitions
        # 0:64, head B on 64:128.  q@k.T runs as two row-tiled K=64 matmuls
        # (PE row-groups 0:63 / 64:127) writing the two halves of one PSUM
        # tile; identical deps make them issue back-to-back and execute
        # CONCURRENTLY on disjoint row-groups - full-array activity at half
        # the matmul slots.
        q_s = main.tile([P, T], MMDT, name="q_s")
        k_s = main.tile([P, T], MMDT, name="k_s")
        v_t = main.tile([P, T], MMDT)
        # v_aug padded to 128 columns (cols 0:64 v, col 64 ones, rest zero)
        v_augA = main.tile([P, 16, P], MMDT, name="v_augA")
        v_augB = main.tile([P, 16, P], MMDT, name="v_augB")
        v_augs = (v_augA, v_augB)
        nc.vector.memset(v_augA, 0.0)
        nc.vector.memset(v_augB, 0.0)
        ones_sb = consts.tile([P, 16, 1], F32)
        nc.vector.memset(ones_sb, 1.0)
        nc.vector.tensor_copy(v_augA[:, :, HD:HD + 1], ones_sb)
        nc.vector.tensor_copy(v_augB[:, :, HD:HD + 1], ones_sb)
        Y = main.tile([P, 8, 2 * P], MMDT)  # [tau_part, mo, r_local]
        Y5 = Y.rearrange("p mo (l d two) -> p mo l d two", l=2, d=HD)

        def qkv_store(g, ps, ts):
            dst = (q_s, k_s, v_t)[g]
            nc.vector.tensor_scalar_add(dst[:, ts], ps, bqkv_s[:, g:g + 1])

        def emit_vtr(nt):
            for b in range(4 * nt, 4 * nt + 4):
                tp = ps_q.tile([P, 512], MMDT, name="ps_tr", tag="mm")
                nc.tensor.transpose(
                    tp[:, 0:P], v_t[:, P * b:P * (b + 1)], identity
                )
                nc.vector.tensor_copy(v_augA[:, b, 0:HD], tp[:, 0:HD])
                nc.vector.tensor_copy(v_augB[:, b, 0:HD], tp[:, HD:2 * HD])

        def emit_qkv0():
            """nt=0 hybrid: f-major over the first 4 x chunks (matmuls start
            as soon as chunk 0 lands), then g-sequential completion so q and
            k finish early and attention block 0 starts sooner."""
            with nc.named_scope("qkv0"):
                ps_g = [
                    ps_att.tile([P, 1024], F32, name="ps_g0", tag="att")[:, 0:512],
                    ps_att.tile([P, 1024], F32, name="ps_g1", tag="att")[:, 0:512],
                    ps_q.tile([P, 512], F32, name="ps_g2", tag="mm"),
                ]
                for f in range(8):
                    for g in range(3):
                        nc.tensor.matmul(
                            ps_g[g],
                            lhsT=wqkv_s[:, f, P * g:P * (g + 1)],
                            rhs=xT_s[:, 0, 512 * f:512 * (f + 1)],
                            start=(f == 0),
                            stop=(f == 7),
                        )
                for g in range(3):
                    qkv_store(g, ps_g[g], slice(0, 512))
                emit_vtr(0)

        def emit_qkv(nt):
            """nt>=1, g-major."""
            with nc.named_scope(f"qkv{nt}"):
                ts = slice(512 * nt, 512 * (nt + 1))
                for g in range(3):
                    ps = ps_q.tile([P, 512], F32, name="ps_mm", tag="mm")
                    for f in range(8):
                        nc.tensor.matmul(
                            ps,
                            lhsT=wqkv_s[:, f, P * g:P * (g + 1)],
                            rhs=xT_s[:, nt, 512 * f:512 * (f + 1)],
                            start=(f == 0),
                            stop=(f == 7),
                        )
                    qkv_store(g, ps, ts)
                emit_vtr(nt)

        proj_partials = []
        proj1_ps = {}

        def p1_mm(mt, nt2, mo):
            key = (mt, nt2)
            if key not in proj1_ps:
                proj1_ps[key] = ps_q.tile([P, 512], F32, name="ps_pr", tag="mm")
            nc.tensor.matmul(
                proj1_ps[key],
                lhsT=Y[:, mo, P * mt:P * (mt + 1)],
                rhs=wproj_s[:, mo, 512 * nt2:512 * (nt2 + 1)],
                start=(mo == 0),
                stop=(mo == 3),
            )

        def p1_spill(mt, nt2, projp):
            part = projp.tile([P, 512], F32, name=f"part{mt}{nt2}", tag="pp")
            # fold the output bias in at spill time
            nc.vector.tensor_add(part, proj1_ps.pop((mt, nt2)),
                                 bproj_bc[:, 512 * nt2:512 * (nt2 + 1)])
            proj_partials.append(((mt, nt2), part))

        def gen_qkv(nt):
            """generator variant of emit_qkv: one unit per g-group / vtr."""
            ts = slice(512 * nt, 512 * (nt + 1))
            for g in range(3):
                with nc.named_scope(f"qkv{nt}"):
                    ps = ps_q.tile([P, 512], F32, name="ps_mm", tag="mm")
                    for f in range(8):
                        nc.tensor.matmul(
                            ps,
                            lhsT=wqkv_s[:, f, P * g:P * (g + 1)],
                            rhs=xT_s[:, nt, 512 * f:512 * (f + 1)],
                            start=(f == 0),
                            stop=(f == 7),
                        )
                    qkv_store(g, ps, ts)
                yield
            with nc.named_scope(f"qkv{nt}"):
                emit_vtr(nt)
            yield

        def gen_attn(bq, expp, ytp, smallp, extra_after_pair=None,
                     after_l=None):
            """attention for t_q block bq; av runs one block behind qk so
            the exp latency hides behind the next block's matmuls."""
            nbk = 4 * bq + 4
            y_ps = [
                ps_y.tile([P, 512], F32, name=f"y_ps{l}", tag="y")
                for l in range(HPC)
            ]
            es_q = []

            def emit_av(bk, es):
                jj = max(0, bk - 4 * bq)
                for l in range(HPC):
                    nc.tensor.matmul(
                        y_ps[l][:, P * jj:512],
                        lhsT=v_augs[l][:, bk, :],
                        rhs=es[:, 512 * l + P * jj:512 * (l + 1)],
                        start=(bk == 0),
                        stop=(bk == nbk - 1),
                        skip_group_check=True,
                    )

            for bk in range(nbk):
                with nc.named_scope(f"attn{bq}"):
                    # one [128, 1024] PSUM tile per k-block holds BOTH
                    # heads' scores; the two K=64 row-tiled q@k matmuls have
                    # identical deps, issue back-to-back, and run
                    # concurrently on PE row-groups 0:63 / 64:127
                    att = ps_att.tile([P, 1024], F32, name="att", tag="att")
                    jj = max(0, bk - 4 * bq)
                    diag = bk >= 4 * bq
                    bsl = slice(P * bk, P * (bk + 1))
                    qsl = slice(512 * bq + P * jj, 512 * (bq + 1))
                    for l in range(HPC):
                        nc.tensor.matmul(
                            att[:, 512 * l + P * jj:512 * (l + 1)],
                            lhsT=k_s[HD * l:HD * (l + 1), bsl],
                            rhs=q_s[HD * l:HD * (l + 1), qsl],
                            start=True, stop=True,
                        )
                    es = expp.tile([P, 1024], MMDT, name="es", tag="es")
                    if not diag:
                        nc.scalar.activation(es, att, EXPF)
                    else:
                        if jj == 0:
                            # both heads' valid ranges are full: one act
                            nc.scalar.activation(es, att, EXPF)
                        else:
                            for l in range(HPC):
                                c0 = 512 * l + P * jj
                                c1 = 512 * (l + 1)
                                nc.scalar.activation(
                                    es[:, c0:c1], att[:, c0:c1], EXPF)
                        for l in range(HPC):
                            c0 = 512 * l + P * jj
                            # zero strictly-upper triangle of the 128x128
                            # diagonal sub-block: keep where f - p >= 0
                            nc.gpsimd.affine_select(
                                out=es[:, c0:c0 + P],
                                in_=es[:, c0:c0 + P],
                                compare_op=mybir.AluOpType.is_ge,
                                fill=0.0,
                                base=0,
                                pattern=[[1, P]],
                                channel_multiplier=-1,
                            )
                    es_q.append((bk, es))
                    if len(es_q) > 2:
                        emit_av(*es_q.pop(0))
                    if extra_after_pair is not None and bk in extra_after_pair:
                        for cb in extra_after_pair[bk]:
                            cb()
                yield
            with nc.named_scope(f"attn{bq}"):
                while es_q:
                    emit_av(*es_q.pop(0))
            yield
            # y_T -> y_nat, normalize, scatter into Y
            for l in range(HPC):
                with nc.named_scope(f"attn{bq}"):
                    yts = ytp.tile([HD + 1, 512], MMDT, name="yts", tag="yts")
                    nc.vector.tensor_copy(yts, y_ps[l][0:HD + 1, :])
                    for sub in range(4):
                        typ = ps_q.tile([P, 512], MMDT, name="ps_ty", tag="mm")
                        nc.tensor.transpose(
                            typ[:, 0:HD + 1],
                            yts[:, P * sub:P * (sub + 1)],
                            identity[0:HD + 1, 0:HD + 1],
                        )
                        rc = smallp.tile([P, 1], F32, name="rc", tag="rc")
                        nc.vector.reciprocal(rc, typ[:, HD:HD + 1])
                        tb = 4 * bq + sub
                        phalf, mo = tb // 8, tb % 8
                        nc.vector.tensor_scalar_mul(
                            Y5[:, mo, l, :, phalf], typ[:, 0:HD], rc
                        )
                    if after_l is not None and l in after_l:
                        after_l[l]()
                yield

        def interleave(attn_gen, n_attn, qkv_gen, n_qkv):
            """merge: spread the qkv units evenly across the attn units."""
            done = 0
            for i in range(n_attn):
                if next(attn_gen, StopIteration) is StopIteration:
                    break
                want = (i + 1) * n_qkv // n_attn
                while done < want:
                    if next(qkv_gen, StopIteration) is StopIteration:
                        done = n_qkv
                        break
                    done += 1
            for _ in attn_gen:
                pass
            for _ in qkv_gen:
                pass

        with (
            tc.tile_pool(name="expp", bufs=8) as expp,
            tc.tile_pool(name="ytp", bufs=2) as ytp,
            tc.tile_pool(name="smallp", bufs=4) as smallp,
            tc.tile_pool(name="projp", bufs=4) as projp,
            tc.tile_pool(name="outp", bufs=2) as outp,
        ):
            emit_qkv0()
            interleave(gen_attn(0, expp, ytp, smallp), 7, gen_qkv(1), 4)
            interleave(gen_attn(1, expp, ytp, smallp), 11, gen_qkv(2), 4)
            interleave(gen_attn(2, expp, ytp, smallp), 15, gen_qkv(3), 4)
            # proj first-half (tau chunks 0..3, ready after attn2) spread as
            # single matmuls across attn3's blocks to keep PE array activity
            # high through the Scalar-bound stretch; second half of
            # output-row block mt only needs head mt's normalize, so it is
            # emitted right after that head's normalize inside attn3
            from functools import partial
            extras = {}
            for ti, (mt, nt2) in enumerate(((0, 0), (0, 1), (1, 0), (1, 1))):
                for mo in range(4):
                    extras[4 * ti + mo] = [partial(p1_mm, mt, nt2, mo)]
                extras[4 * ti + 3].append(partial(p1_spill, mt, nt2, projp))

            def emit_proj_part2(mt):
                for (mt_, nt2), part in proj_partials:
                    if mt_ != mt:
                        continue
                    ps = ps_q.tile([P, 512], F32, name="ps_pr2", tag="mm")
                    for mo in range(4, 8):
                        nc.tensor.matmul(
                            ps,
                            lhsT=Y[:, mo, P * mt:P * (mt + 1)],
                            rhs=wproj_s[:, mo, 512 * nt2:512 * (nt2 + 1)],
                            start=(mo == 4),
                            stop=(mo == 7),
                        )
                    os_ = outp.tile([P, 512], F32, name="os", tag="os")
                    nc.vector.tensor_add(os_, ps, part)
                    nc.sync.dma_start(
                        out=out_d.ap()[P * mt:P * (mt + 1),
                                       512 * nt2:512 * (nt2 + 1)],
                        in_=os_,
                    )

            for _ in gen_attn(3, expp, ytp, smallp, extra_after_pair=extras,
                              after_l={0: lambda: emit_proj_part2(0),
                                       1: lambda: emit_proj_part2(1)}):
                pass

    nc.compile()
    return nc


_nc_cache = None


def kernel(**inputs):
    global _nc_cache
    x = np.asarray(inputs["x"], dtype=np.float32)
    w_attn = np.asarray(inputs["w_attn"], dtype=np.float32)
    b_attn = np.asarray(inputs["b_attn"], dtype=np.float32)
    w_proj = np.asarray(inputs["w_proj"], dtype=np.float32)
    b_proj = np.asarray(inputs["b_proj"], dtype=np.float32)

    scale = 1.0 / math.sqrt(HD)
    # xq[p, qtr, fo*512 + t] = x[512*qtr + t, 128*fo + p]
    xq = _to_mm(
        x.reshape(4, 512, 8, P).transpose(3, 0, 2, 1).reshape(P, 4, 8 * 512)
    )
    # wproj[p, mo*1024 + n] = w_proj[n, 128*mo + p]
    wproj_h = _to_mm(
        w_proj.T.reshape(8, P, C).transpose(1, 0, 2).reshape(P, 8 * C)
    )
    bproj_r = np.ascontiguousarray(b_proj[None, :])

    in_maps = []
    for c in range(NCORES):
        ch0 = P * c
        wq = w_attn[ch0:ch0 + P, :] * scale
        wk = w_attn[C + ch0:C + ch0 + P, :]
        wv = w_attn[2 * C + ch0:2 * C + ch0 + P, :]
        wstack = np.concatenate([wq, wk, wv], axis=0)  # [384, 1024]
        # wqkv[p, f*384 + col] = wstack[col, 128*f + p]
        wqkv_h = _to_mm(
            wstack.T.reshape(8, P, 384).transpose(1, 0, 2).reshape(P, 8 * 384)
        )
        bqkv = np.ascontiguousarray(
            np.stack(
                [
                    b_attn[ch0:ch0 + P] * scale,
                    b_attn[C + ch0:C + ch0 + P],
                    b_attn[2 * C + ch0:2 * C + ch0 + P],
                ],
                axis=1,
            )
        )
        in_maps.append(
            {
                "xq": xq,
                "wqkv": wqkv_h,
                "bqkv": bqkv,
                "wproj": wproj_h,
                "bproj": bproj_r,
            }
        )

    if _nc_cache is None:
        _nc_cache = build_nc()
    nc = _nc_cache

    trace = os.environ.get("BASS_KERNEL_TRACE", "0") == "1"
    res = bass_utils.run_bass_kernel_spmd(
        nc, in_maps, core_ids=list(range(NCORES)), trace=trace
    )
    if trace:
        print(f"HW exec time: {res.exec_time_ns} ns")
        if res.per_core_scope_times:
            for scope, times in sorted(res.per_core_scope_times.items()):
                print(f"  scope {scope}: {times}")
        if res.instructions_and_trace:
            print(f"  trace: {res.instructions_and_trace[1]}")

    out = np.concatenate([r["out"] for r in res.results], axis=0)
    return np.ascontiguousarray(out.astype(np.float32))


if __name__ == "__main__":
    nc = build_nc()
    print("build OK")


# revision 47
# speedup vs baseline: 1.0079x; 1.0079x over previous
"""Trainium2 Bass kernel for causal self-attention (T=2048, C=1024, NH=16).

Strategy (tensor-parallel over heads, 2 heads/core on 8 cores):
  - Host pre-packs x, w_attn-slice, w_proj into the exact SBUF layouts so
    every DMA is a [128, N] fully-contiguous transfer (cheap descriptor
    generation; strided rearranges cost ~3us each on the Sync engine).
  - DMA issue order follows need order: wqkv first, then x quarter 0 as 8
    per-fo chunks (the first qkv matmul group only needs chunk 0), then
    the rest; wproj (needed last) is issued last.
  - qkv block nt=0 runs f-major with three concurrent PSUM accumulators so
    matmuls start as soon as the first x chunk lands; later blocks run
    g-major.  Attention block bq is emitted one qkv block behind so the
    Scalar-engine exp work overlaps qkv matmuls.
  - All matmuls use the full 128x128 PE array (q/k zero-padded per head,
    v padded to 128 columns): the PE HAM clock-gate watches array
    *activity*; low-occupancy matmuls read as idle and re-throttle the
    clock to 4/8.
  - att_T = k @ q.T tiles ([t_k, t_q]) so softmax's denominator comes for
    free from an appended ones-column on v.  Softmax without
    max-subtraction (inputs bounded, |att| < 8).  exp runs only on the
    causally-valid column range of diagonal tiles; the per-block
    triangular mask is applied in-place on GpSimd (affine_select), and
    the q@k / att@v streams skip fully-masked columns.
  - The reference's bug-faithful reshape (NH,T,HD)->swap(1,2)->(T,C) makes
    the output row-parallel over heads: each core produces 256 full output
    rows; host concatenates, no collective.
  - The output projection is split: the first half of its contraction
    (tau chunks 0..3, complete after attention block 2) is computed during
    the Scalar-bound attention block 3 and spilled to SBUF, keeping the
    PE busy; the rest runs after.
"""
import math
import os

import numpy as np

import concourse.bass as bass
import concourse.bacc as bacc
import concourse.mybir as mybir
import concourse.tile as tile
from concourse import bass_utils
from concourse.masks import make_identity

T, C, NH, HD = 2048, 1024, 16, 64
P = 128
NCORES = 8
HPC = 2          # heads per core
F32 = mybir.dt.float32
MMDT = mybir.dt.bfloat16  # matmul input dtype
EXPF = mybir.ActivationFunctionType.Exp


def _to_mm(a):
    import ml_dtypes
    return np.ascontiguousarray(np.asarray(a, dtype=np.float32).astype(ml_dtypes.bfloat16))


def build_nc():
    nc = bacc.Bacc(trn_type="TRN2", target_bir_lowering=False)

    xq_d = nc.dram_tensor("xq", [P, 4, 8 * 512], MMDT, kind="ExternalInput")
    wqkv_d = nc.dram_tensor("wqkv", [P, 8 * 384], MMDT, kind="ExternalInput")
    bqkv_d = nc.dram_tensor("bqkv", [P, 3], F32, kind="ExternalInput")
    wproj_d = nc.dram_tensor("wproj", [P, 8 * 1024], MMDT, kind="ExternalInput")
    bproj_d = nc.dram_tensor("bproj", [1, C], F32, kind="ExternalInput")
    out_d = nc.dram_tensor("out", [2 * P, C], F32, kind="ExternalOutput")

    from contextlib import ExitStack

    with tile.TileContext(nc) as tc, ExitStack() as stack:
        consts = stack.enter_context(tc.tile_pool(name="consts", bufs=1))
        wpool = stack.enter_context(tc.tile_pool(name="wpool", bufs=1))
        main = stack.enter_context(tc.tile_pool(name="main", bufs=1))
        ps_q = stack.enter_context(tc.tile_pool(name="ps_q", bufs=2, space="PSUM"))
        ps_att = stack.enter_context(tc.tile_pool(name="ps_att", bufs=2, space="PSUM"))
        ps_y = stack.enter_context(tc.tile_pool(name="ps_y", bufs=2, space="PSUM"))

        # ---- DMAs split across both HWDGE queues (Sync + Scalar), issued
        # in need order; x quarter 1 gets 4 queue slots so it lands before
        # qkv block 1 needs it ----
        wqkv_s = wpool.tile([P, 8, 384], MMDT)
        xT_s = wpool.tile([P, 4, 8 * 512], MMDT)
        xr = xq_d.ap()
        wr = wqkv_d.ap().rearrange("p (f c) -> p f c", f=8)
        for lo, hi in ((0, 1), (1, 2), (2, 4), (4, 6), (6, 8)):
            nc.scalar.dma_start(out=wqkv_s[:, lo:hi, :], in_=wr[:, lo:hi, :])
            nc.sync.dma_start(out=xT_s[:, 0, 512 * lo:512 * hi],
                              in_=xr[:, 0, 512 * lo:512 * hi])
        for i in range(4):
            eng = nc.sync if i % 2 == 0 else nc.scalar
            eng.dma_start(out=xT_s[:, 1, 1024 * i:1024 * (i + 1)],
                          in_=xr[:, 1, 1024 * i:1024 * (i + 1)])
        bqkv_s = consts.tile([P, 3], F32)
        nc.scalar.dma_start(out=bqkv_s, in_=bqkv_d.ap())
        for qtr in range(2, 4):
            nc.sync.dma_start(out=xT_s[:, qtr, 0:2048], in_=xr[:, qtr, 0:2048])
            nc.scalar.dma_start(out=xT_s[:, qtr, 2048:4096], in_=xr[:, qtr, 2048:4096])
        wproj_s = wpool.tile([P, 8, 1024], MMDT)
        nc.sync.dma_start(out=wproj_s, in_=wproj_d.ap())
        bproj_bc = consts.tile([P, C], F32)
        bp = bproj_d.ap()
        bp_bcast = bass.AP(tensor=bp.tensor, offset=bp.offset,
                           ap=[[0, P]] + list(bp.ap[1:]))
        nc.scalar.dma_start(out=bproj_bc, in_=bp_bcast)

        # ---- constants / padded state ----
        with nc.named_scope("setup"):
            # PE warm-up: the HAM clock gate starts at K=4/8 and needs
            # ~3.4us of sustained full-array activity to release; burn that
            # time on zero matmuls while the input DMAs are still in flight
            # so the real matmuls start at full clock
            wu = consts.tile([P, 640], MMDT)
            nc.vector.memset(wu, 0.0)
            ps_wu = ps_q.tile([P, 512], F32, name="ps_wu", tag="mm")
            for i in range(8):
                nc.tensor.matmul(
                    ps_wu,
                    lhsT=wu[:, 0:P],
                    rhs=wu[:, P:640],
                    start=(i == 0),
                    stop=(i == 7),
                )
            # bf16 identity: keeps every PE transpose in bf16 so the fp32-HI
            # FWL-disable workaround never triggers
            identity = consts.tile([P, P], MMDT)
            make_identity(nc, identity)

        # q/k for both heads in single tiles: head A channels on partitions
        # 0:64, head B on 64:128.  q@k.T runs as two row-tiled K=64 matmuls
        # (PE row-groups 0:63 / 64:127) writing the two halves of one PSUM
        # tile; identical deps make them issue back-to-back and execute
        # CONCURRENTLY on disjoint row-groups - full-array activity at half
        # the matmul slots.
        q_s = main.tile([P, T], MMDT, name="q_s")
        k_s = main.tile([P, T], MMDT, name="k_s")
        v_t = main.tile([P, T], MMDT)
        # v_aug padded to 128 columns (cols 0:64 v, col 64 ones, rest zero)
        v_augA = main.tile([P, 16, P], MMDT, name="v_augA")
        v_augB = main.tile([P, 16, P], MMDT, name="v_augB")
        v_augs = (v_augA, v_augB)
        nc.vector.memset(v_augA, 0.0)
        nc.vector.memset(v_augB, 0.0)
        ones_sb = consts.tile([P, 16, 1], F32)
        nc.vector.memset(ones_sb, 1.0)
        nc.vector.tensor_copy(v_augA[:, :, HD:HD + 1], ones_sb)
        nc.vector.tensor_copy(v_augB[:, :, HD:HD + 1], ones_sb)
        Y = main.tile([P, 8, 2 * P], MMDT)  # [tau_part, mo, r_local]
        Y5 = Y.rearrange("p mo (l d two) -> p mo l d two", l=2, d=HD)

        def qkv_store(g, ps, ts):
            dst = (q_s, k_s, v_t)[g]
            nc.vector.tensor_scalar_add(dst[:, ts], ps, bqkv_s[:, g:g + 1])

        def emit_vtr(nt):
            for b in range(4 * nt, 4 * nt + 4):
                tp = ps_q.tile([P, 512], MMDT, name="ps_tr", tag="mm")
                nc.tensor.transpose(
                    tp[:, 0:P], v_t[:, P * b:P * (b + 1)], identity
                )
                nc.vector.tensor_copy(v_augA[:, b, 0:HD], tp[:, 0:HD])
                nc.vector.tensor_copy(v_augB[:, b, 0:HD], tp[:, HD:2 * HD])

        def emit_qkv0():
            """nt=0 hybrid: f-major over the first 4 x chunks (matmuls start
            as soon as chunk 0 lands), then g-sequential completion so q and
            k finish early and attention block 0 starts sooner."""
            with nc.named_scope("qkv0"):
                ps_g = [
                    ps_att.tile([P, 1024], F32, name="ps_g0", tag="att")[:, 0:512],
                    ps_att.tile([P, 1024], F32, name="ps_g1", tag="att")[:, 0:512],
                    ps_q.tile([P, 512], F32, name="ps_g2", tag="mm"),
                ]
                for f in range(8):
                    for g in range(3):
                        nc.tensor.matmul(
                            ps_g[g],
                            lhsT=wqkv_s[:, f, P * g:P * (g + 1)],
                            rhs=xT_s[:, 0, 512 * f:512 * (f + 1)],
                            start=(f == 0),
                            stop=(f == 7),
                        )
                for g in range(3):
                    qkv_store(g, ps_g[g], slice(0, 512))
                # filler: keep the PE array active (HAM) while Vector runs
                # the three bias-adds that gate attention block 0
                ps_f = ps_y.tile([P, 512], F32, name="ps_fill", tag="y")
                for i in range(6):
                    nc.tensor.matmul(
                        ps_f,
                        lhsT=wu[:, 0:P],
                        rhs=wu[:, P:640],
                        start=(i == 0),
                        stop=(i == 5),
                    )
                emit_vtr(0)

        def emit_qkv(nt):
            """nt>=1, g-major."""
            with nc.named_scope(f"qkv{nt}"):
                ts = slice(512 * nt, 512 * (nt + 1))
                for g in range(3):
                    ps = ps_q.tile([P, 512], F32, name="ps_mm", tag="mm")
                    for f in range(8):
                        nc.tensor.matmul(
                            ps,
                            lhsT=wqkv_s[:, f, P * g:P * (g + 1)],
                            rhs=xT_s[:, nt, 512 * f:512 * (f + 1)],
                            start=(f == 0),
                            stop=(f == 7),
                        )
                    qkv_store(g, ps, ts)
                emit_vtr(nt)

        proj_partials = []
        proj1_ps = {}

        def p1_mm(mt, nt2, mo):
            key = (mt, nt2)
            if key not in proj1_ps:
                proj1_ps[key] = ps_q.tile([P, 512], F32, name="ps_pr", tag="mm")
            nc.tensor.matmul(
                proj1_ps[key],
                lhsT=Y[:, mo, P * mt:P * (mt + 1)],
                rhs=wproj_s[:, mo, 512 * nt2:512 * (nt2 + 1)],
                start=(mo == 0),
                stop=(mo == 3),
            )

        def p1_spill(mt, nt2, projp):
            part = projp.tile([P, 512], F32, name=f"part{mt}{nt2}", tag="pp")
            # fold the output bias in at spill time
            nc.vector.tensor_add(part, proj1_ps.pop((mt, nt2)),
                                 bproj_bc[:, 512 * nt2:512 * (nt2 + 1)])
            proj_partials.append(((mt, nt2), part))

        def gen_qkv(nt):
            """generator variant of emit_qkv: one unit per g-group / vtr."""
            ts = slice(512 * nt, 512 * (nt + 1))
            for g in range(3):
                with nc.named_scope(f"qkv{nt}"):
                    ps = ps_q.tile([P, 512], F32, name="ps_mm", tag="mm")
                    for f in range(8):
                        nc.tensor.matmul(
                            ps,
                            lhsT=wqkv_s[:, f, P * g:P * (g + 1)],
                            rhs=xT_s[:, nt, 512 * f:512 * (f + 1)],
                            start=(f == 0),
                            stop=(f == 7),
                        )
                    qkv_store(g, ps, ts)
                yield
            with nc.named_scope(f"qkv{nt}"):
                emit_vtr(nt)
            yield

        def gen_attn(bq, expp, ytp, smallp, extra_after_pair=None,
                     after_l=None):
            """attention for t_q block bq; av runs one block behind qk so
            the exp latency hides behind the next block's matmuls."""
            nbk = 4 * bq + 4
            y_ps = [
                ps_y.tile([P, 512], F32, name=f"y_ps{l}", tag="y")
                for l in range(HPC)
            ]
            es_q = []

            def emit_av(bk, es):
                jj = max(0, bk - 4 * bq)
                for l in range(HPC):
                    nc.tensor.matmul(
                        y_ps[l][:, P * jj:512],
                        lhsT=v_augs[l][:, bk, :],
                        rhs=es[:, 512 * l + P * jj:512 * (l + 1)],
                        start=(bk == 0),
                        stop=(bk == nbk - 1),
                        skip_group_check=True,
                    )

            for bk in range(nbk):
                with nc.named_scope(f"attn{bq}"):
                    # one [128, 1024] PSUM tile per k-block holds BOTH
                    # heads' scores; the two K=64 row-tiled q@k matmuls have
                    # identical deps, issue back-to-back, and run
                    # concurrently on PE row-groups 0:63 / 64:127
                    att = ps_att.tile([P, 1024], F32, name="att", tag="att")
                    jj = max(0, bk - 4 * bq)
                    diag = bk >= 4 * bq
                    bsl = slice(P * bk, P * (bk + 1))
                    qsl = slice(512 * bq + P * jj, 512 * (bq + 1))
                    for l in range(HPC):
                        nc.tensor.matmul(
                            att[:, 512 * l + P * jj:512 * (l + 1)],
                            lhsT=k_s[HD * l:HD * (l + 1), bsl],
                            rhs=q_s[HD * l:HD * (l + 1), qsl],
                            start=True, stop=True,
                        )
                    es = expp.tile([P, 1024], MMDT, name="es", tag="es")
                    if not diag:
                        nc.scalar.activation(es, att, EXPF)
                    else:
                        if jj == 0:
                            # both heads' valid ranges are full: one act
                            nc.scalar.activation(es, att, EXPF)
                        else:
                            for l in range(HPC):
                                c0 = 512 * l + P * jj
                                c1 = 512 * (l + 1)
                                nc.scalar.activation(
                                    es[:, c0:c1], att[:, c0:c1], EXPF)
                        for l in range(HPC):
                            c0 = 512 * l + P * jj
                            # zero strictly-upper triangle of the 128x128
                            # diagonal sub-block: keep where f - p >= 0
                            nc.gpsimd.affine_select(
                                out=es[:, c0:c0 + P],
                                in_=es[:, c0:c0 + P],
                                compare_op=mybir.AluOpType.is_ge,
                                fill=0.0,
                                base=0,
                                pattern=[[1, P]],
                                channel_multiplier=-1,
                            )
                    es_q.append((bk, es))
                    if len(es_q) > 2:
                        emit_av(*es_q.pop(0))
                    if extra_after_pair is not None and bk in extra_after_pair:
                        for cb in extra_after_pair[bk]:
                            cb()
                yield
            with nc.named_scope(f"attn{bq}"):
                while es_q:
                    emit_av(*es_q.pop(0))
            yield
            # y_T -> y_nat, normalize, scatter into Y
            for l in range(HPC):
                with nc.named_scope(f"attn{bq}"):
                    yts = ytp.tile([HD + 1, 512], MMDT, name="yts", tag="yts")
                    nc.vector.tensor_copy(yts, y_ps[l][0:HD + 1, :])
                    for sub in range(4):
                        typ = ps_q.tile([P, 512], MMDT, name="ps_ty", tag="mm")
                        nc.tensor.transpose(
                            typ[:, 0:HD + 1],
                            yts[:, P * sub:P * (sub + 1)],
                            identity[0:HD + 1, 0:HD + 1],
                        )
                        rc = smallp.tile([P, 1], F32, name="rc", tag="rc")
                        nc.vector.reciprocal(rc, typ[:, HD:HD + 1])
                        tb = 4 * bq + sub
                        phalf, mo = tb // 8, tb % 8
                        nc.vector.tensor_scalar_mul(
                            Y5[:, mo, l, :, phalf], typ[:, 0:HD], rc
                        )
                    if after_l is not None and l in after_l:
                        after_l[l]()
                yield

        def interleave(attn_gen, n_attn, qkv_gen, n_qkv):
            """merge: spread the qkv units evenly across the attn units."""
            done = 0
            for i in range(n_attn):
                if next(attn_gen, StopIteration) is StopIteration:
                    break
                want = (i + 1) * n_qkv // n_attn
                while done < want:
                    if next(qkv_gen, StopIteration) is StopIteration:
                        done = n_qkv
                        break
                    done += 1
            for _ in attn_gen:
                pass
            for _ in qkv_gen:
                pass

        with (
            tc.tile_pool(name="expp", bufs=8) as expp,
            tc.tile_pool(name="ytp", bufs=2) as ytp,
            tc.tile_pool(name="smallp", bufs=4) as smallp,
            tc.tile_pool(name="projp", bufs=4) as projp,
            tc.tile_pool(name="outp", bufs=2) as outp,
        ):
            emit_qkv0()
            interleave(gen_attn(0, expp, ytp, smallp), 7, gen_qkv(1), 4)
            interleave(gen_attn(1, expp, ytp, smallp), 11, gen_qkv(2), 4)
            interleave(gen_attn(2, expp, ytp, smallp), 15, gen_qkv(3), 4)
            # proj first-half (tau chunks 0..3, ready after attn2) spread as
            # single matmuls across attn3's blocks to keep PE array activity
            # high through the Scalar-bound stretch; second half of
            # output-row block mt only needs head mt's normalize, so it is
            # emitted right after that head's normalize inside attn3
            from functools import partial
            extras = {}
            for ti, (mt, nt2) in enumerate(((0, 0), (0, 1), (1, 0), (1, 1))):
                for mo in range(4):
                    extras[4 * ti + mo] = [partial(p1_mm, mt, nt2, mo)]
                extras[4 * ti + 3].append(partial(p1_spill, mt, nt2, projp))

            def emit_proj_part2(mt):
                for (mt_, nt2), part in proj_partials:
                    if mt_ != mt:
                        continue
                    ps = ps_q.tile([P, 512], F32, name="ps_pr2", tag="mm")
                    for mo in range(4, 8):
                        nc.tensor.matmul(
                            ps,
                            lhsT=Y[:, mo, P * mt:P * (mt + 1)],
                            rhs=wproj_s[:, mo, 512 * nt2:512 * (nt2 + 1)],
                            start=(mo == 4),
                            stop=(mo == 7),
                        )
                    os_ = outp.tile([P, 512], F32, name="os", tag="os")
                    nc.vector.tensor_add(os_, ps, part)
                    nc.sync.dma_start(
                        out=out_d.ap()[P * mt:P * (mt + 1),
                                       512 * nt2:512 * (nt2 + 1)],
                        in_=os_,
                    )

            for _ in gen_attn(3, expp, ytp, smallp, extra_after_pair=extras,
                              after_l={0: lambda: emit_proj_part2(0),
                                       1: lambda: emit_proj_part2(1)}):
                pass

    nc.compile()
    return nc


_nc_cache = None


def kernel(**inputs):
    global _nc_cache
    x = np.asarray(inputs["x"], dtype=np.float32)
    w_attn = np.asarray(inputs["w_attn"], dtype=np.float32)
    b_attn = np.asarray(inputs["b_attn"], dtype=np.float32)
    w_proj = np.asarray(inputs["w_proj"], dtype=np.float32)
    b_proj = np.asarray(inputs["b_proj"], dtype=np.float32)

    scale = 1.0 / math.sqrt(HD)
    # xq[p, qtr, fo*512 + t] = x[512*qtr + t, 128*fo + p]
    xq = _to_mm(
        x.reshape(4, 512, 8, P).transpose(3, 0, 2, 1).reshape(P, 4, 8 * 512)
    )
    # wproj[p, mo*1024 + n] = w_proj[n, 128*mo + p]
    wproj_h = _to_mm(
        w_proj.T.reshape(8, P, C).transpose(1, 0, 2).reshape(P, 8 * C)
    )
    bproj_r = np.ascontiguousarray(b_proj[None, :])

    in_maps = []
    for c in range(NCORES):
        ch0 = P * c
        wq = w_attn[ch0:ch0 + P, :] * scale
        wk = w_attn[C + ch0:C + ch0 + P, :]
        wv = w_attn[2 * C + ch0:2 * C + ch0 + P, :]
        wstack = np.concatenate([wq, wk, wv], axis=0)  # [384, 1024]
        # wqkv[p, f*384 + col] = wstack[col, 128*f + p]
        wqkv_h = _to_mm(
            wstack.T.reshape(8, P, 384).transpose(1, 0, 2).reshape(P, 8 * 384)
        )
        bqkv = np.ascontiguousarray(
            np.stack(
                [
                    b_attn[ch0:ch0 + P] * scale,
                    b_attn[C + ch0:C + ch0 + P],
                    b_attn[2 * C + ch0:2 * C + ch0 + P],
                ],
                axis=1,
            )
        )
        in_maps.append(
            {
                "xq": xq,
                "wqkv": wqkv_h,
                "bqkv": bqkv,
                "wproj": wproj_h,
                "bproj": bproj_r,
            }
        )

    if _nc_cache is None:
        _nc_cache = build_nc()
    nc = _nc_cache

    trace = os.environ.get("BASS_KERNEL_TRACE", "0") == "1"
    res = bass_utils.run_bass_kernel_spmd(
        nc, in_maps, core_ids=list(range(NCORES)), trace=trace
    )
    if trace:
        print(f"HW exec time: {res.exec_time_ns} ns")
        if res.per_core_scope_times:
            for scope, times in sorted(res.per_core_scope_times.items()):
                print(f"  scope {scope}: {times}")
        if res.instructions_and_trace:
            print(f"  trace: {res.instructions_and_trace[1]}")

    out = np.concatenate([r["out"] for r in res.results], axis=0)
    return np.ascontiguousarray(out.astype(np.float32))


if __name__ == "__main__":
    nc = build_nc()
    print("build OK")


# revision 48
# speedup vs baseline: 1.0404x; 1.0322x over previous
"""Trainium2 Bass kernel for causal self-attention (T=2048, C=1024, NH=16).

Strategy (tensor-parallel over heads, 2 heads/core on 8 cores):
  - Host pre-packs x, w_attn-slice, w_proj into the exact SBUF layouts so
    every DMA is a [128, N] fully-contiguous transfer (cheap descriptor
    generation; strided rearranges cost ~3us each on the Sync engine).
  - DMA issue order follows need order: wqkv first, then x quarter 0 as 8
    per-fo chunks (the first qkv matmul group only needs chunk 0), then
    the rest; wproj (needed last) is issued last.
  - qkv block nt=0 runs f-major with three concurrent PSUM accumulators so
    matmuls start as soon as the first x chunk lands; later blocks run
    g-major.  Attention block bq is emitted one qkv block behind so the
    Scalar-engine exp work overlaps qkv matmuls.
  - All matmuls use the full 128x128 PE array (q/k zero-padded per head,
    v padded to 128 columns): the PE HAM clock-gate watches array
    *activity*; low-occupancy matmuls read as idle and re-throttle the
    clock to 4/8.
  - att_T = k @ q.T tiles ([t_k, t_q]) so softmax's denominator comes for
    free from an appended ones-column on v.  Softmax without
    max-subtraction (inputs bounded, |att| < 8).  exp runs only on the
    causally-valid column range of diagonal tiles; the per-block
    triangular mask is applied in-place on GpSimd (affine_select), and
    the q@k / att@v streams skip fully-masked columns.
  - The reference's bug-faithful reshape (NH,T,HD)->swap(1,2)->(T,C) makes
    the output row-parallel over heads: each core produces 256 full output
    rows; host concatenates, no collective.
  - The output projection is split: the first half of its contraction
    (tau chunks 0..3, complete after attention block 2) is computed during
    the Scalar-bound attention block 3 and spilled to SBUF, keeping the
    PE busy; the rest runs after.
"""
import math
import os

import numpy as np

import concourse.bass as bass
import concourse.bacc as bacc
import concourse.mybir as mybir
import concourse.tile as tile
from concourse import bass_utils
from concourse.masks import make_identity

T, C, NH, HD = 2048, 1024, 16, 64
P = 128
NCORES = 8
HPC = 2          # heads per core
F32 = mybir.dt.float32
MMDT = mybir.dt.bfloat16  # matmul input dtype
EXPF = mybir.ActivationFunctionType.Exp


def _to_mm(a):
    import ml_dtypes
    return np.ascontiguousarray(np.asarray(a, dtype=np.float32).astype(ml_dtypes.bfloat16))


def build_nc():
    nc = bacc.Bacc(trn_type="TRN2", target_bir_lowering=False)

    xq_d = nc.dram_tensor("xq", [P, 4, 8 * 512], MMDT, kind="ExternalInput")
    wqkv_d = nc.dram_tensor("wqkv", [P, 8 * 384], MMDT, kind="ExternalInput")
    bqkv_d = nc.dram_tensor("bqkv", [P, 3], F32, kind="ExternalInput")
    wproj_d = nc.dram_tensor("wproj", [P, 8 * 1024], MMDT, kind="ExternalInput")
    bproj_d = nc.dram_tensor("bproj", [1, C], F32, kind="ExternalInput")
    out_d = nc.dram_tensor("out", [2 * P, C], F32, kind="ExternalOutput")

    from contextlib import ExitStack

    with tile.TileContext(nc) as tc, ExitStack() as stack:
        consts = stack.enter_context(tc.tile_pool(name="consts", bufs=1))
        wpool = stack.enter_context(tc.tile_pool(name="wpool", bufs=1))
        main = stack.enter_context(tc.tile_pool(name="main", bufs=1))
        ps_q = stack.enter_context(tc.tile_pool(name="ps_q", bufs=2, space="PSUM"))
        ps_att = stack.enter_context(tc.tile_pool(name="ps_att", bufs=2, space="PSUM"))
        ps_y = stack.enter_context(tc.tile_pool(name="ps_y", bufs=2, space="PSUM"))

        # ---- DMAs split across both HWDGE queues (Sync + Scalar), issued
        # in need order; x quarter 1 gets 4 queue slots so it lands before
        # qkv block 1 needs it ----
        wqkv_s = wpool.tile([P, 8, 384], MMDT)
        xT_s = wpool.tile([P, 4, 8 * 512], MMDT)
        xr = xq_d.ap()
        wr = wqkv_d.ap().rearrange("p (f c) -> p f c", f=8)
        for lo, hi in ((0, 1), (1, 2), (2, 4), (4, 6), (6, 8)):
            nc.scalar.dma_start(out=wqkv_s[:, lo:hi, :], in_=wr[:, lo:hi, :])
            nc.sync.dma_start(out=xT_s[:, 0, 512 * lo:512 * hi],
                              in_=xr[:, 0, 512 * lo:512 * hi])
        for i in range(4):
            eng = nc.sync if i % 2 == 0 else nc.scalar
            eng.dma_start(out=xT_s[:, 1, 1024 * i:1024 * (i + 1)],
                          in_=xr[:, 1, 1024 * i:1024 * (i + 1)])
        bqkv_s = consts.tile([P, 3], F32)
        nc.scalar.dma_start(out=bqkv_s, in_=bqkv_d.ap())
        for qtr in range(2, 4):
            nc.sync.dma_start(out=xT_s[:, qtr, 0:2048], in_=xr[:, qtr, 0:2048])
            nc.scalar.dma_start(out=xT_s[:, qtr, 2048:4096], in_=xr[:, qtr, 2048:4096])
        wproj_s = wpool.tile([P, 8, 1024], MMDT)
        nc.sync.dma_start(out=wproj_s, in_=wproj_d.ap())
        bproj_bc = consts.tile([P, C], F32)
        bp = bproj_d.ap()
        bp_bcast = bass.AP(tensor=bp.tensor, offset=bp.offset,
                           ap=[[0, P]] + list(bp.ap[1:]))
        nc.scalar.dma_start(out=bproj_bc, in_=bp_bcast)

        # ---- constants / padded state ----
        with nc.named_scope("setup"):
            # PE warm-up: the HAM clock gate starts at K=4/8 and needs
            # ~3.4us of sustained full-array activity to release; burn that
            # time on zero matmuls while the input DMAs are still in flight
            # so the real matmuls start at full clock
            wu = consts.tile([P, 640], MMDT)
            nc.vector.memset(wu, 0.0)
            ps_wu = ps_q.tile([P, 512], F32, name="ps_wu", tag="mm")
            for i in range(8):
                nc.tensor.matmul(
                    ps_wu,
                    lhsT=wu[:, 0:P],
                    rhs=wu[:, P:640],
                    start=(i == 0),
                    stop=(i == 7),
                )
            # bf16 identity: keeps every PE transpose in bf16 so the fp32-HI
            # FWL-disable workaround never triggers
            identity = consts.tile([P, P], MMDT)
            make_identity(nc, identity)

        # q/k for both heads in single tiles: head A channels on partitions
        # 0:64, head B on 64:128.  q@k.T runs as two row-tiled K=64 matmuls
        # (PE row-groups 0:63 / 64:127) writing the two halves of one PSUM
        # tile; identical deps make them issue back-to-back and execute
        # CONCURRENTLY on disjoint row-groups - full-array activity at half
        # the matmul slots.
        q_s = main.tile([P, T], MMDT, name="q_s")
        k_s = main.tile([P, T], MMDT, name="k_s")
        v_t = main.tile([P, T], MMDT)
        # v_aug padded to 128 columns (cols 0:64 v, col 64 ones, rest zero)
        v_augA = main.tile([P, 16, P], MMDT, name="v_augA")
        v_augB = main.tile([P, 16, P], MMDT, name="v_augB")
        v_augs = (v_augA, v_augB)
        nc.vector.memset(v_augA, 0.0)
        nc.vector.memset(v_augB, 0.0)
        ones_sb = consts.tile([P, 16, 1], F32)
        nc.vector.memset(ones_sb, 1.0)
        nc.vector.tensor_copy(v_augA[:, :, HD:HD + 1], ones_sb)
        nc.vector.tensor_copy(v_augB[:, :, HD:HD + 1], ones_sb)
        Y = main.tile([P, 8, 2 * P], MMDT)  # [tau_part, mo, r_local]
        Y5 = Y.rearrange("p mo (l d two) -> p mo l d two", l=2, d=HD)

        def qkv_store(g, ps, ts):
            dst = (q_s, k_s, v_t)[g]
            nc.vector.tensor_scalar_add(dst[:, ts], ps, bqkv_s[:, g:g + 1])

        def emit_vtr(nt):
            for b in range(4 * nt, 4 * nt + 4):
                tp = ps_q.tile([P, 512], MMDT, name="ps_tr", tag="mm")
                nc.tensor.transpose(
                    tp[:, 0:P], v_t[:, P * b:P * (b + 1)], identity
                )
                nc.vector.tensor_copy(v_augA[:, b, 0:HD], tp[:, 0:HD])
                nc.vector.tensor_copy(v_augB[:, b, 0:HD], tp[:, HD:2 * HD])

        def emit_qkv0():
            """nt=0 hybrid: f-major over the first 4 x chunks (matmuls start
            as soon as chunk 0 lands), then g-sequential completion so q and
            k finish early and attention block 0 starts sooner."""
            with nc.named_scope("qkv0"):
                ps_g = [
                    ps_att.tile([P, 1024], F32, name="ps_g0", tag="att")[:, 0:512],
                    ps_att.tile([P, 1024], F32, name="ps_g1", tag="att")[:, 0:512],
                    ps_q.tile([P, 512], F32, name="ps_g2", tag="mm"),
                ]
                for f in range(8):
                    for g in range(3):
                        nc.tensor.matmul(
                            ps_g[g],
                            lhsT=wqkv_s[:, f, P * g:P * (g + 1)],
                            rhs=xT_s[:, 0, 512 * f:512 * (f + 1)],
                            start=(f == 0),
                            stop=(f == 7),
                        )
                for g in range(3):
                    qkv_store(g, ps_g[g], slice(0, 512))
                # filler: keep the PE array active (HAM) while Vector runs
                # the three bias-adds that gate attention block 0
                ps_f = ps_y.tile([P, 512], F32, name="ps_fill", tag="y")
                for i in range(6):
                    nc.tensor.matmul(
                        ps_f,
                        lhsT=wu[:, 0:P],
                        rhs=wu[:, P:640],
                        start=(i == 0),
                        stop=(i == 5),
                    )
                emit_vtr(0)

        def emit_qkv(nt):
            """nt>=1, g-major."""
            with nc.named_scope(f"qkv{nt}"):
                ts = slice(512 * nt, 512 * (nt + 1))
                for g in range(3):
                    ps = ps_q.tile([P, 512], F32, name="ps_mm", tag="mm")
                    for f in range(8):
                        nc.tensor.matmul(
                            ps,
                            lhsT=wqkv_s[:, f, P * g:P * (g + 1)],
                            rhs=xT_s[:, nt, 512 * f:512 * (f + 1)],
                            start=(f == 0),
                            stop=(f == 7),
                        )
                    qkv_store(g, ps, ts)
                emit_vtr(nt)

        proj_partials = []
        proj1_ps = {}

        def p1_mm(mt, nt2, mo):
            key = (mt, nt2)
            if key not in proj1_ps:
                proj1_ps[key] = ps_q.tile([P, 512], F32, name="ps_pr", tag="mm")
            nc.tensor.matmul(
                proj1_ps[key],
                lhsT=Y[:, mo, P * mt:P * (mt + 1)],
                rhs=wproj_s[:, mo, 512 * nt2:512 * (nt2 + 1)],
                start=(mo == 0),
                stop=(mo == 3),
            )

        def p1_spill(mt, nt2, projp):
            part = projp.tile([P, 512], F32, name=f"part{mt}{nt2}", tag="pp")
            # fold the output bias in at spill time
            nc.vector.tensor_add(part, proj1_ps.pop((mt, nt2)),
                                 bproj_bc[:, 512 * nt2:512 * (nt2 + 1)])
            proj_partials.append(((mt, nt2), part))

        def gen_qkv(nt):
            """generator variant of emit_qkv: one unit per g-group / vtr."""
            ts = slice(512 * nt, 512 * (nt + 1))
            for g in range(3):
                with nc.named_scope(f"qkv{nt}"):
                    ps = ps_q.tile([P, 512], F32, name="ps_mm", tag="mm")
                    for f in range(8):
                        nc.tensor.matmul(
                            ps,
                            lhsT=wqkv_s[:, f, P * g:P * (g + 1)],
                            rhs=xT_s[:, nt, 512 * f:512 * (f + 1)],
                            start=(f == 0),
                            stop=(f == 7),
                        )
                    qkv_store(g, ps, ts)
                yield
            with nc.named_scope(f"qkv{nt}"):
                emit_vtr(nt)
            yield

        def gen_attn(bq, expp, ytp, smallp, extra_after_pair=None,
                     after_l=None):
            """attention for t_q block bq; av runs one block behind qk so
            the exp latency hides behind the next block's matmuls."""
            nbk = 4 * bq + 4
            y_ps = [
                ps_y.tile([P, 512], F32, name=f"y_ps{l}", tag="y")
                for l in range(HPC)
            ]
            es_q = []

            def emit_av(bk, es):
                jj = max(0, bk - 4 * bq)
                for l in range(HPC):
                    nc.tensor.matmul(
                        y_ps[l][:, P * jj:512],
                        lhsT=v_augs[l][:, bk, :],
                        rhs=es[:, 512 * l + P * jj:512 * (l + 1)],
                        start=(bk == 0),
                        stop=(bk == nbk - 1),
                        skip_group_check=True,
                    )

            for bk in range(nbk):
                with nc.named_scope(f"attn{bq}"):
                    # one [128, 1024] PSUM tile per k-block holds BOTH
                    # heads' scores; the two K=64 row-tiled q@k matmuls have
                    # identical deps, issue back-to-back, and run
                    # concurrently on PE row-groups 0:63 / 64:127
                    att = ps_att.tile([P, 1024], F32, name="att", tag="att")
                    jj = max(0, bk - 4 * bq)
                    diag = bk >= 4 * bq
                    bsl = slice(P * bk, P * (bk + 1))
                    qsl = slice(512 * bq + P * jj, 512 * (bq + 1))
                    for l in range(HPC):
                        nc.tensor.matmul(
                            att[:, 512 * l + P * jj:512 * (l + 1)],
                            lhsT=k_s[HD * l:HD * (l + 1), bsl],
                            rhs=q_s[HD * l:HD * (l + 1), qsl],
                            start=True, stop=True,
                        )
                    es = expp.tile([P, 1024], MMDT, name="es", tag="es")
                    if not diag:
                        nc.scalar.activation(es, att, EXPF)
                    else:
                        if jj == 0:
                            # both heads' valid ranges are full: one act
                            nc.scalar.activation(es, att, EXPF)
                        else:
                            for l in range(HPC):
                                c0 = 512 * l + P * jj
                                c1 = 512 * (l + 1)
                                nc.scalar.activation(
                                    es[:, c0:c1], att[:, c0:c1], EXPF)
                        for l in range(HPC):
                            c0 = 512 * l + P * jj
                            # zero strictly-upper triangle of the 128x128
                            # diagonal sub-block: keep where f - p >= 0
                            nc.gpsimd.affine_select(
                                out=es[:, c0:c0 + P],
                                in_=es[:, c0:c0 + P],
                                compare_op=mybir.AluOpType.is_ge,
                                fill=0.0,
                                base=0,
                                pattern=[[1, P]],
                                channel_multiplier=-1,
                            )
                    es_q.append((bk, es))
                    if len(es_q) > 2:
                        emit_av(*es_q.pop(0))
                    if extra_after_pair is not None and bk in extra_after_pair:
                        for cb in extra_after_pair[bk]:
                            cb()
                yield
            with nc.named_scope(f"attn{bq}"):
                while es_q:
                    emit_av(*es_q.pop(0))
            yield
            # y_T -> y_nat, normalize, scatter into Y
            for l in range(HPC):
                with nc.named_scope(f"attn{bq}"):
                    yts = ytp.tile([HD + 1, 512], MMDT, name="yts", tag="yts")
                    nc.vector.tensor_copy(yts, y_ps[l][0:HD + 1, :])
                    for sub in range(4):
                        typ = ps_q.tile([P, 512], MMDT, name="ps_ty", tag="mm")
                        nc.tensor.transpose(
                            typ[:, 0:HD + 1],
                            yts[:, P * sub:P * (sub + 1)],
                            identity[0:HD + 1, 0:HD + 1],
                        )
                        rc = smallp.tile([P, 1], F32, name="rc", tag="rc")
                        nc.vector.reciprocal(rc, typ[:, HD:HD + 1])
                        tb = 4 * bq + sub
                        phalf, mo = tb // 8, tb % 8
                        nc.vector.tensor_scalar_mul(
                            Y5[:, mo, l, :, phalf], typ[:, 0:HD], rc
                        )
                    if after_l is not None and l in after_l:
                        after_l[l]()
                yield

        def interleave(attn_gen, n_attn, qkv_gen, n_qkv):
            """merge: spread the qkv units evenly across the attn units."""
            done = 0
            for i in range(n_attn):
                if next(attn_gen, StopIteration) is StopIteration:
                    break
                want = (i + 1) * n_qkv // n_attn
                while done < want:
                    if next(qkv_gen, StopIteration) is StopIteration:
                        done = n_qkv
                        break
                    done += 1
            for _ in attn_gen:
                pass
            for _ in qkv_gen:
                pass

        with (
            tc.tile_pool(name="expp", bufs=8) as expp,
            tc.tile_pool(name="ytp", bufs=2) as ytp,
            tc.tile_pool(name="smallp", bufs=4) as smallp,
            tc.tile_pool(name="projp", bufs=4) as projp,
            tc.tile_pool(name="outp", bufs=4) as outp,
        ):
            emit_qkv0()
            interleave(gen_attn(0, expp, ytp, smallp), 7, gen_qkv(1), 4)
            interleave(gen_attn(1, expp, ytp, smallp), 11, gen_qkv(2), 4)
            interleave(gen_attn(2, expp, ytp, smallp), 15, gen_qkv(3), 4)
            # proj first-half (tau chunks 0..3, ready after attn2) spread as
            # single matmuls across attn3's blocks to keep PE array activity
            # high through the Scalar-bound stretch; second half of
            # output-row block mt only needs head mt's normalize, so it is
            # emitted right after that head's normalize inside attn3
            from functools import partial
            extras = {}
            for ti, (mt, nt2) in enumerate(((0, 0), (0, 1), (1, 0), (1, 1))):
                for mo in range(4):
                    extras[4 * ti + mo] = [partial(p1_mm, mt, nt2, mo)]
                extras[4 * ti + 3].append(partial(p1_spill, mt, nt2, projp))

            def emit_proj_part2(mt):
                for (mt_, nt2), part in proj_partials:
                    if mt_ != mt:
                        continue
                    ps = ps_q.tile([P, 512], F32, name="ps_pr2", tag="mm")
                    for mo in range(4, 8):
                        nc.tensor.matmul(
                            ps,
                            lhsT=Y[:, mo, P * mt:P * (mt + 1)],
                            rhs=wproj_s[:, mo, 512 * nt2:512 * (nt2 + 1)],
                            start=(mo == 4),
                            stop=(mo == 7),
                        )
                    os_ = outp.tile([P, 512], F32, name="os", tag="os")
                    nc.vector.tensor_add(os_, ps, part)
                    nc.sync.dma_start(
                        out=out_d.ap()[P * mt:P * (mt + 1),
                                       512 * nt2:512 * (nt2 + 1)],
                        in_=os_,
                    )

            for _ in gen_attn(3, expp, ytp, smallp, extra_after_pair=extras,
                              after_l={0: lambda: emit_proj_part2(0),
                                       1: lambda: emit_proj_part2(1)}):
                pass

    nc.compile()
    return nc


_nc_cache = None


def kernel(**inputs):
    global _nc_cache
    x = np.asarray(inputs["x"], dtype=np.float32)
    w_attn = np.asarray(inputs["w_attn"], dtype=np.float32)
    b_attn = np.asarray(inputs["b_attn"], dtype=np.float32)
    w_proj = np.asarray(inputs["w_proj"], dtype=np.float32)
    b_proj = np.asarray(inputs["b_proj"], dtype=np.float32)

    scale = 1.0 / math.sqrt(HD)
    # xq[p, qtr, fo*512 + t] = x[512*qtr + t, 128*fo + p]
    xq = _to_mm(
        x.reshape(4, 512, 8, P).transpose(3, 0, 2, 1).reshape(P, 4, 8 * 512)
    )
    # wproj[p, mo*1024 + n] = w_proj[n, 128*mo + p]
    wproj_h = _to_mm(
        w_proj.T.reshape(8, P, C).transpose(1, 0, 2).reshape(P, 8 * C)
    )
    bproj_r = np.ascontiguousarray(b_proj[None, :])

    in_maps = []
    for c in range(NCORES):
        ch0 = P * c
        wq = w_attn[ch0:ch0 + P, :] * scale
        wk = w_attn[C + ch0:C + ch0 + P, :]
        wv = w_attn[2 * C + ch0:2 * C + ch0 + P, :]
        wstack = np.concatenate([wq, wk, wv], axis=0)  # [384, 1024]
        # wqkv[p, f*384 + col] = wstack[col, 128*f + p]
        wqkv_h = _to_mm(
            wstack.T.reshape(8, P, 384).transpose(1, 0, 2).reshape(P, 8 * 384)
        )
        bqkv = np.ascontiguousarray(
            np.stack(
                [
                    b_attn[ch0:ch0 + P] * scale,
                    b_attn[C + ch0:C + ch0 + P],
                    b_attn[2 * C + ch0:2 * C + ch0 + P],
                ],
                axis=1,
            )
        )
        in_maps.append(
            {
                "xq": xq,
                "wqkv": wqkv_h,
                "bqkv": bqkv,
                "wproj": wproj_h,
                "bproj": bproj_r,
            }
        )

    if _nc_cache is None:
        _nc_cache = build_nc()
    nc = _nc_cache

    trace = os.environ.get("BASS_KERNEL_TRACE", "0") == "1"
    res = bass_utils.run_bass_kernel_spmd(
        nc, in_maps, core_ids=list(range(NCORES)), trace=trace
    )
    if trace:
        print(f"HW exec time: {res.exec_time_ns} ns")
        if res.per_core_scope_times:
            for scope, times in sorted(res.per_core_scope_times.items()):
                print(f"  scope {scope}: {times}")
        if res.instructions_and_trace:
            print(f"  trace: {res.instructions_and_trace[1]}")

    out = np.concatenate([r["out"] for r in res.results], axis=0)
    return np.ascontiguousarray(out.astype(np.float32))


if __name__ == "__main__":
    nc = build_nc()
    print("build OK")
